# revision 9
# baseline (speedup 1.0000x reference)
"""Multi-head attention (B=2, S=2048, D=1024, H=16) on 8 Trainium2 cores.

Sharding: core c -> batch b = c // 4, head group g = c % 4 (4 heads each).
Each core computes its 4 heads end-to-end (QKV proj -> attention -> out-proj
partials) and returns two partial [S, D] outputs (one per head pair); the
host sums the 8 partials per batch and adds bv@Wo + bo (the V-bias folds out
of attention exactly: softmax rows sum to 1, so attn @ (V+bv) = attn@V + bv).

Design notes (from trace analysis of earlier versions):
  - The PE streams matmuls at ~1 col/cycle only when the full 128x128 array
    is engaged. A lone K=64 matmul runs at half rate, but TWO K=64 matmuls
    on disjoint row halves (tile_position (0,0)/(64,0)) stream concurrently:
    the scores for a head pair are issued back-to-back that way, costing
    ~one N=512 stream for both. M=65 matmuls also run at half rate: the PV
    matmul pads V~ to 128 columns (V | ones | zeros) -> full rate, and the
    ones column still yields the softmax denominator in U row 64 for free.
  - All operands fp16 (halves DMA + SBUF; matmul rate is dtype-independent;
    error budget is 2e-2, fp16 keeps us ~1e-3).
  - V is built directly in [keys, feat] layout with X^T tiles as lhsT (no
    PE transposes), interleaved per key tile into the first attention round
    so it rides under the exp chain instead of delaying it.
  - Exp is ScalarE-only at ~(N+352)/1.2 ns: one ACT per (head pair, q block,
    key tile) over [128, 1024] covering both heads; the 128 ACTs (~138us)
    run back-to-back on double-buffered s tiles - the near-critical chain.
  - Engine queues are strict FIFO, so emission order is the schedule:
    inputs are DMA'd on both HW queues (sync + scalar) in consumption
    order; head-pair 0 is projected first so the exp chain starts ~20us in;
    head-pair 1's projections ride in the hp0 attention phase; each round's
    out-proj partial is emitted inside the NEXT round's kt loop so it fills
    PE slack instead of stalling on the norm at round boundaries.
  - PSUM budget (8 banks): scores s-tiles [128,1024]f32 x2 bufs = 4,
    shared work pool [128,512]f32 x2 = 2, U accumulators x2 heads = 2.
"""

import numpy as np
from contextlib import ExitStack

import concourse.bass as bass
import concourse.mybir as mybir
import concourse.tile as tile
from concourse import bacc
from concourse.bass import ts, ds
from concourse.bass_utils import run_bass_kernel_spmd

F32 = mybir.dt.float32
F16 = mybir.dt.float16

B, S, D = 2, 2048, 1024
H_TOT, HD = 16, 64
HC = 4                 # heads per core
DC = HC * HD           # 256 columns of QKV proj per core
NCORES = 8
P = 128
NDT = D // P           # 8 d-model tiles
NKT = S // P           # 16 key tiles
QB = 512               # q block width in attention
NQB = S // QB          # 4 q blocks
XC = 1024              # x chunk width for projections
NXC = S // XC          # 2 x chunks
SCALE = 1.0 / np.sqrt(HD)


def _body(ctx, tc, xq, xk, xv, wq, wk, wv, bq, bk, wo, outp):
    nc = tc.nc

    singles = ctx.enter_context(tc.tile_pool(name="singles", bufs=1))
    ppool = ctx.enter_context(tc.tile_pool(name="ppool", bufs=4))
    opool = ctx.enter_context(tc.tile_pool(name="opool", bufs=2))
    psS = ctx.enter_context(tc.tile_pool(name="psS", bufs=2, space="PSUM"))
    psW = ctx.enter_context(tc.tile_pool(name="psW", bufs=2, space="PSUM"))
    psU = ctx.enter_context(tc.tile_pool(name="psU", bufs=1, space="PSUM"))

    # Persistent per-core tensors (partition dim x free dims)
    XA = [singles.tile([P, NDT, XC], F16, tag=f"xa{c}", name=f"xa{c}") for c in range(NXC)]
    XB = [singles.tile([P, NDT, XC], F16, tag=f"xb{c}", name=f"xb{c}") for c in range(NXC)]
    XV = singles.tile([P, NDT, S], F16, tag="xv")
    QT = [singles.tile([P, S], F16, tag=f"qt{m}", name=f"qt{m}") for m in range(2)]
    KT = [singles.tile([P, S], F16, tag=f"kt{m}", name=f"kt{m}") for m in range(2)]
    OT = [singles.tile([P, S], F16, tag=f"ot{m}", name=f"ot{m}") for m in range(2)]
    Vt = singles.tile([P, NKT, HC, P], F16, tag="vtile")  # [keys, kt, h, 128]

    # V~ pad: ones in col 64 (denominator), zeros in cols 65..127.
    nc.vector.memset(Vt[:, :, :, HD + 1 :], 0.0)
    nc.vector.memset(Vt[:, :, :, HD : HD + 1], 1.0)

    # DMAs on the sync (SP) HW queue, in consumption order
    wk_sb = singles.tile([P, NDT, DC], F16, tag="wk")
    nc.sync.dma_start(out=wk_sb, in_=wk.rearrange("(t p) c -> p t c", p=P))
    bk_sb = singles.tile([P, 2], F32, tag="bk")
    nc.sync.dma_start(out=bk_sb, in_=bk.rearrange("(m p) -> p m", p=P))
    wq_sb = singles.tile([P, NDT, DC], F16, tag="wq")
    nc.sync.dma_start(out=wq_sb, in_=wq.rearrange("(t p) c -> p t c", p=P))
    bq_sb = singles.tile([P, 2], F32, tag="bq")
    nc.sync.dma_start(out=bq_sb, in_=bq.rearrange("(m p) -> p m", p=P))
    for cg in range(NXC):
        nc.sync.dma_start(
            out=XB[cg], in_=xk[:, ts(cg, XC)].rearrange("(t p) q -> p t q", p=P)
        )
    for cg in range(NXC):
        nc.sync.dma_start(
            out=XA[cg], in_=xq[:, ts(cg, XC)].rearrange("(t p) q -> p t q", p=P)
        )
    wo_sb = singles.tile([P, 2, D], F16, tag="wo")
    nc.sync.dma_start(out=wo_sb, in_=wo.rearrange("(k p) d -> p k d", p=P))

    # xv + wv on the scalar (Activation) HW queue, concurrent with the above
    wv_sb = singles.tile([P, NDT, DC], F16, tag="wv")
    nc.scalar.dma_start(out=wv_sb, in_=wv.rearrange("(t p) c -> p t c", p=P))
    nc.scalar.dma_start(out=XV, in_=xv.rearrange("(t p) q -> p t q", p=P))

    def kq_proj(m):
        """K/Q projections of head pair m: W^T @ X^T, feat-on-partition."""
        for w_sb, b_sb, xset, DEST in (
            (wk_sb, bk_sb, XB, KT),
            (wq_sb, bq_sb, XA, QT),
        ):
            for cg in range(NXC):
                for c2 in range(2):
                    ps = psW.tile([P, 512], F32, tag="mm")
                    for dt in range(NDT):
                        nc.tensor.matmul(
                            ps,
                            lhsT=w_sb[:, dt, ts(m, P)],
                            rhs=xset[cg][:, dt, ts(c2, 512)],
                            start=(dt == 0),
                            stop=(dt == NDT - 1),
                        )
                    nc.vector.tensor_scalar_add(
                        out=DEST[m][:, ds(cg * XC + c2 * 512, 512)],
                        in0=ps,
                        scalar1=b_sb[:, m : m + 1],
                    )

    def vproj(kt):
        """V tile kt in [keys, feat] layout: lhsT = X^T slab, rhs = Wv."""
        ps = psW.tile([P, 512], F32, tag="mm")
        for dt in range(NDT):
            nc.tensor.matmul(
                ps[:, 0:DC],
                lhsT=XV[:, dt, ts(kt, P)],
                rhs=wv_sb[:, dt, :],
                start=(dt == 0),
                stop=(dt == NDT - 1),
            )
        nc.vector.tensor_copy(
            out=Vt[:, kt, :, 0:HD],
            in_=ps[:, 0:DC].rearrange("p (h d) -> p h d", d=HD),
        )

    def outproj(hp, qb):
        """Out-proj partial for head pair hp's rows of Wo, q block qb."""
        for qt in range(qb * QB // P, (qb + 1) * QB // P):
            for c2 in range(2):
                ps = psW.tile([P, 512], F32, tag="mm")
                nc.tensor.matmul(
                    ps,
                    lhsT=OT[hp][:, ts(qt, P)],
                    rhs=wo_sb[:, hp, ts(c2, 512)],
                    start=True,
                    stop=True,
                )
                ob = opool.tile([P, 512], F16, tag="ob", bufs=3)
                nc.vector.tensor_copy(out=ob, in_=ps)
                nc.sync.dma_start(out=outp[hp, ts(qt, P), ts(c2, 512)], in_=ob)

    pending = []  # deferred emissions that fill the next round's PE slack

    def attn_round(hp, qb, with_vproj=False):
        U = [psU.tile([P, QB], F32, tag=f"u{i}", name=f"u{i}") for i in range(2)]
        pe_prev = None
        for kt in range(NKT):
            s = psS.tile([P, 2 * QB], F32, tag="s")
            for i in range(2):
                po = 64 * i
                nc.tensor.matmul(
                    s[:, ts(i, QB)],
                    lhsT=KT[hp][po : po + 64, ts(kt, P)],
                    rhs=QT[hp][po : po + 64, ts(qb, QB)],
                    start=True,
                    stop=True,
                    tile_position=(po, 0),
                )
            if with_vproj:
                vproj(kt)
            if kt == 2:
                while pending:
                    pending.pop(0)()
            if pe_prev is not None:
                for i in range(2):
                    nc.tensor.matmul(
                        U[i],
                        lhsT=Vt[:, kt - 1, 2 * hp + i, :],
                        rhs=pe_prev[:, ts(i, QB)],
                        start=(kt == 1),
                        stop=False,
                    )
            pe = ppool.tile([P, 2 * QB], F16, tag="pexp")
            nc.scalar.activation(
                out=pe, in_=s,
                func=mybir.ActivationFunctionType.Exp,
                scale=float(SCALE),
            )
            pe_prev = pe
        for i in range(2):
            nc.tensor.matmul(
                U[i],
                lhsT=Vt[:, NKT - 1, 2 * hp + i, :],
                rhs=pe_prev[:, ts(i, QB)],
                start=False,
                stop=True,
            )

        # softmax denominators: evacuate U rows 0..64 to SBUF (frees the
        # PSUM bank), broadcast the sum row across partitions, reciprocal
        for i in range(2):
            usb = opool.tile([HD + 1, QB], F32, tag="ou", bufs=3)
            nc.vector.tensor_copy(out=usb, in_=U[i][0 : HD + 1, :])
            bc = opool.tile([64, QB], F32, tag="bc", bufs=2)
            nc.gpsimd.partition_broadcast(bc, usb[HD : HD + 1, :])
            nc.vector.reciprocal_approx_fast(out=bc, in_=bc)
            nc.vector.tensor_mul(
                out=OT[hp][64 * i : 64 * i + 64, ts(qb, QB)],
                in0=usb[0:HD, :],
                in1=bc,
            )
        pending.append(lambda hp=hp, qb=qb: outproj(hp, qb))

    # head pair 0: project k/q, then its attention rounds (v-proj rides in
    # the first round); head pair 1's projections go between the phases
    kq_proj(0)
    attn_round(0, 0, with_vproj=True)
    for qb in range(1, NQB):
        attn_round(0, qb)
    kq_proj(1)
    for qb in range(NQB):
        attn_round(1, qb)
    while pending:
        pending.pop(0)()


def build_nc():
    nc = bacc.Bacc("TRN2", target_bir_lowering=False, debug=False)
    aps = {}
    for name, shape, dt_ in (
        ("xq", [D, S], F16),
        ("xk", [D, S], F16),
        ("xv", [D, S], F16),
        ("wq", [D, DC], F16),
        ("wk", [D, DC], F16),
        ("wv", [D, DC], F16),
        ("bq", [DC], F32),
        ("bk", [DC], F32),
        ("wo", [DC, D], F16),
    ):
        aps[name] = nc.dram_tensor(name, shape, dt_, kind="ExternalInput").ap()
    aps["outp"] = nc.dram_tensor("out_partial", [2, S, D], F16, kind="ExternalOutput").ap()

    with tile.TileContext(nc) as tc:
        with ExitStack() as ctx:
            _body(
                ctx,
                tc,
                aps["xq"], aps["xk"], aps["xv"],
                aps["wq"], aps["wk"], aps["wv"],
                aps["bq"], aps["bk"],
                aps["wo"], aps["outp"],
            )
    nc.compile()
    return nc


def make_in_maps(inputs):
    q = np.asarray(inputs["query"], dtype=np.float32)
    k = np.asarray(inputs.get("key_", inputs.get("key")), dtype=np.float32)
    v = np.asarray(inputs["value"], dtype=np.float32)
    Wq = np.asarray(inputs["Wq"], dtype=np.float16)
    Wk = np.asarray(inputs["Wk"], dtype=np.float16)
    Wv = np.asarray(inputs["Wv"], dtype=np.float16)
    bq = np.asarray(inputs["bq"], dtype=np.float32)
    bk = np.asarray(inputs["bk"], dtype=np.float32)
    Wo = np.asarray(inputs["Wo"], dtype=np.float16)

    # one host transpose per batch, shared by the 4 cores of that batch
    qT = [np.ascontiguousarray(q[b].T.astype(np.float16)) for b in range(B)]
    kT = [np.ascontiguousarray(k[b].T.astype(np.float16)) for b in range(B)]
    vT = [np.ascontiguousarray(v[b].T.astype(np.float16)) for b in range(B)]

    in_maps = []
    for c in range(NCORES):
        b, g = divmod(c, 4)
        cs = slice(DC * g, DC * (g + 1))
        in_maps.append(
            {
                "xq": qT[b],
                "xk": kT[b],
                "xv": vT[b],
                "wq": np.ascontiguousarray(Wq[:, cs]),
                "wk": np.ascontiguousarray(Wk[:, cs]),
                "wv": np.ascontiguousarray(Wv[:, cs]),
                "bq": np.ascontiguousarray(bq[cs]),
                "bk": np.ascontiguousarray(bk[cs]),
                "wo": np.ascontiguousarray(Wo[cs, :]),
            }
        )
    return in_maps


_NC_CACHE = {}


def get_nc():
    if "nc" not in _NC_CACHE:
        _NC_CACHE["nc"] = build_nc()
    return _NC_CACHE["nc"]


def kernel(**inputs):
    nc = get_nc()
    in_maps = make_in_maps(inputs)
    res = run_bass_kernel_spmd(nc, in_maps, list(range(NCORES))).results
    bo = np.asarray(inputs["bo"], dtype=np.float32)
    bv = np.asarray(inputs["bv"], dtype=np.float32)
    Wo = np.asarray(inputs["Wo"], dtype=np.float32)
    # softmax rows sum to 1, so the V bias folds to a constant output row
    extra = bv @ Wo + bo
    out = np.empty((B, S, D), dtype=np.float32)
    for b in range(B):
        parts = res[4 * b + 0]["out_partial"].astype(np.float32)
        acc = parts[0] + parts[1]
        for g in range(1, 4):
            p = res[4 * b + g]["out_partial"]
            acc = acc + p[0] + p[1]
        out[b] = acc + extra[None, :]
    return out
